# revision 1
# baseline (speedup 1.0000x reference)
"""ContextBranch (context-RoI pooling + 1x1-conv fusion) on 8 Trainium2 cores.

Problem: for each of N=128 boxes, pool the 8 surrounding context cells
(3x3 grid minus center) from a [256, 64, 64] feature map with ROIAlignV2
(7x7 output, sampling_ratio 2), concatenate the 8 pooled chunks into 2048
channels, apply a 1x1 conv (2048->256) + bias + ReLU.

Sharding: box-parallel. Core m handles boxes [16m, 16m+16) and all 8 of
their context cells (128 context boxes per core). The feature map and
fusion weights are replicated.

Device algorithm (per core), built to avoid any on-chip transpose:
  1. ROIAlignV2 is separable: pooled[b] = By_b @ Wnd_b @ Bx_b^T where
     Wnd_b is an 8x8 window of the feature map (a context cell spans
     at most ~4.3 feature pixels + bilinear taps, so 8x8 always covers
     it). Collapsed: pooled[b,s,c] = sum_p M_b[s,p] * Wnd_b[p,c] with
     M_b = By (x) Bx in [49, 64], computed on host from the boxes.
  2. The 64 window pixel vectors of all 128 context boxes are gathered
     on host from a pixel-major bf16 copy of the feature map (the walrus
     build here cannot compile the GPSIMD library reload that the
     on-chip DMAGatherAnt needs) and DMA'd in as 8 np-major chunks with
     channels on partitions: G[c, (pair, b01, p)].
  3. Fusion-first GEMM: F[(b01,p), o] = sum_c G[c,(b01,p)] * w[o, kc]
     (lhsT = gathered window pair, rhs = w slice, accumulate 2 c-halves).
  4. Interp-second GEMM: out[(b01,s), o] += Mbd_pair^T @ F, accumulating
     the 8 context offsets k of a box pair in PSUM (block-diagonal M per
     pair), plus a rank-1 bias row; then ReLU on the scalar engine and
     DMA out as [16, 49, 256] per core.
Host reassembles [128, 49, 256] -> [128, 256, 7, 7].
"""

import numpy as np
import ml_dtypes

import concourse.bass as bass
import concourse.tile as tile
from concourse import mybir
from concourse import bass_utils
from concourse import library_config
from concourse.vector_clock import ScopedClock

# ---------------------------------------------------------------- constants
OUT = 7          # output size
SR = 2           # sampling ratio
SCALE = 1.0 / 16.0
H = W = 64
C = 256
N_BOXES = 128
N_CORES = 8
NB = N_BOXES // N_CORES   # 16 boxes per core
K8 = 8                    # context offsets
NPAIR = NB // 2           # 8 box pairs per core
PAIRS = K8 * NPAIR        # 64 (k, pair) groups per core
WIN = 8                   # window side
WPX = WIN * WIN           # 64 window pixels
S49 = OUT * OUT           # 49 pooled positions
NIDX = PAIRS * 2 * WPX    # 8192 gathered window pixels per core
CHUNK = NIDX // NPAIR     # 1024 pixels per np-chunk

BF16 = ml_dtypes.bfloat16


# ------------------------------------------------------- tile drain patch
def _patched_drain_and_barrier(self, tick_clock, wait_clock):
    # The walrus build in this environment rejects >1 sync wait on a Drain
    # ("Too many sync wait commands"), but Tile's kernel-tail drain carries
    # one wait per live semaphore. Split into chained single-wait drains on
    # the same engine, which is semantically identical.
    nc = self.nc
    drain_bi = nc.sync.drain()
    inst = drain_bi.ins
    wait_clock.add_sem_waits(inst, ScopedClock({None: tick_clock.global_clock}))
    si = inst.sync_info
    waits = list(si.on_wait) if si is not None else []
    if len(waits) > 1:
        inst.sync_info = mybir.SyncInfo(on_wait=[waits[0]], on_update=[])
        for w in waits[1:]:
            d2 = nc.sync.drain()
            d2.ins.sync_info = mybir.SyncInfo(on_wait=[w], on_update=[])

    nc.all_engine_barrier()
    assert self.sems is not None
    popped = nc._tile_sem_poison_stack.pop()
    assert popped is self._sem_poison
    nc.clear_and_free_semaphores(list(self.sems.allocated().values()))
    nc.all_engine_barrier()


tile.TileContext._drain_and_barrier = _patched_drain_and_barrier

_ws_counter = [0]


def _split_multi_waits(nc):
    """Walrus here allows only ONE sync wait per instruction. For every
    instruction carrying N>1 waits, hoist N-1 of them onto injected NoOps on
    the same engine immediately before it (same-engine program order makes
    this semantically identical)."""
    for f in nc.m.functions:
        for blk in f.blocks:
            new_insts = []
            for inst in blk.instructions:
                si = getattr(inst, "sync_info", None)
                waits = list(si.on_wait) if si is not None else []
                if len(waits) > 1:
                    for w in waits[:-1]:
                        _ws_counter[0] += 1
                        nop = mybir.InstNoOp(
                            name=f"I-waitsplit-{_ws_counter[0]}", ins=[], outs=[]
                        )
                        nop.engine = inst.engine
                        nop.sync_info = mybir.SyncInfo(on_wait=[w], on_update=[])
                        nc.register_instruction(nop)
                        new_insts.append(nop)
                    inst.sync_info = mybir.SyncInfo(
                        on_wait=[waits[-1]], on_update=list(si.on_update)
                    )
                new_insts.append(inst)
            blk.instructions = new_insts


# ------------------------------------------------------------- host math
def _context_boxes(boxes):
    """[N,4] -> [8, N, 4] context cells, offset-major (reference order)."""
    boxes = boxes.astype(np.float32)
    x1, y1, x2, y2 = boxes[:, 0], boxes[:, 1], boxes[:, 2], boxes[:, 3]
    w = (x2 - x1) / np.float32(3.0)
    h = (y2 - y1) / np.float32(3.0)
    offs = []
    for i in range(3):
        for j in range(3):
            if i == 1 and j == 1:
                continue
            dx = j * w
            dy = i * h
            offs.append(np.stack([x1 + dx, y1 + dy, x1 + dx + w, y1 + dy + h], axis=1))
    return np.stack(offs, axis=0)


def _axis_weights(lo_c, hi_c, size):
    """Per-axis pooled interp weights for one axis of all B context boxes.

    lo_c, hi_c: [B] box edge coords (image space). Returns (orig [B] int,
    Wax [B, 7, 8] fp32) with pooling (x0.5) folded in.
    """
    B = lo_c.shape[0]
    start = lo_c * np.float32(SCALE) - np.float32(0.5)
    end = hi_c * np.float32(SCALE) - np.float32(0.5)
    bw = (end - start) / np.float32(OUT)
    j = np.arange(OUT * SR)
    t = (j // SR + ((j % SR) + np.float32(0.5)) / np.float32(SR)).astype(np.float32)
    pos = start[:, None] + t[None, :] * bw[:, None]          # [B, 14]
    valid = (pos >= -1.0) & (pos <= size)
    pc = np.clip(pos, np.float32(0.0), np.float32(size - 1))
    lo = np.clip(np.floor(pc), 0.0, size - 2).astype(np.int64)
    f = (pc - lo.astype(np.float32)).astype(np.float32)
    wl = ((1.0 - f) * valid).astype(np.float32)
    wh = (f * valid).astype(np.float32)
    orig = np.clip(lo.min(axis=1), 0, size - WIN)            # [B]
    rel = lo - orig[:, None]                                 # [B, 14] in [0, 6]
    assert rel.min() >= 0 and rel.max() <= WIN - 2
    Wax = np.zeros((B, OUT, WIN), np.float32)
    bi = np.arange(B)
    for jj in range(OUT * SR):
        g = jj // SR
        Wax[bi, g, rel[:, jj]] += 0.5 * wl[:, jj]
        Wax[bi, g, rel[:, jj] + 1] += 0.5 * wh[:, jj]
    return orig, Wax


def _prep(features, boxes, w_fuse, b_fuse):
    """All host-side layout/index prep. Returns (shared dict, per-core list)."""
    features = np.asarray(features, np.float32)
    boxes = np.asarray(boxes, np.float32)
    w_fuse = np.asarray(w_fuse, np.float32)
    b_fuse = np.asarray(b_fuse, np.float32)

    cb = _context_boxes(boxes).reshape(K8 * N_BOXES, 4)      # [1024, 4]
    B = cb.shape[0]
    ox, Wx = _axis_weights(cb[:, 0], cb[:, 2], W)            # x axis
    oy, Wy = _axis_weights(cb[:, 1], cb[:, 3], H)            # y axis

    # M[b, s=(ph,pw), p=(iy,ix)] = Wy[b,ph,iy] * Wx[b,pw,ix]
    M = (Wy[:, :, None, :, None] * Wx[:, None, :, None, :]).reshape(B, S49, WPX)

    # gather pixel index of window pixel p=(iy,ix) of cbox b
    iy, ix = np.meshgrid(np.arange(WIN), np.arange(WIN), indexing="ij")
    pix = ((oy[:, None, None] + iy) * W + (ox[:, None, None] + ix)).reshape(B, WPX)
    assert pix.min() >= 0 and pix.max() < H * W

    # shared tensors
    featT = np.ascontiguousarray(features.reshape(C, H * W).T).astype(BF16)
    w4 = w_fuse.T.reshape(K8, 2, 128, C)                     # [k, c_hi, c_lo, o]
    wsb = np.ascontiguousarray(w4.transpose(2, 0, 1, 3).reshape(128, K8 * 2 * C)).astype(BF16)
    brow = b_fuse.reshape(1, C).astype(BF16)
    ones = np.ones((1, 2 * S49), np.float32).astype(BF16)
    shared = {"wsb": wsb, "brow": brow, "ones": ones}

    # The reference reshapes offset-major pooled [8N,...] to [N, 2048, ...]:
    # output box n is fused from cboxes 8n+kc (kc = chunk group 0..7), i.e.
    # consecutive offset-major indices — NOT box n's own 8 offsets. Core m
    # (boxes 16m..16m+15) therefore consumes cboxes [128m, 128m+128).
    per_core = []
    for m in range(N_CORES):
        mbd = np.zeros((PAIRS, 128, 2 * S49), np.float32)
        idx = np.zeros((PAIRS, 2, WPX), np.int64)
        for kc in range(K8):
            for np_ in range(NPAIR):
                # np-major pair order: gather chunk np_ = pairs [8np_, 8np_+8)
                # covers all 8 chunk-groups of one box pair, so each outer
                # np_ iteration depends on exactly one gather chunk.
                pair = np_ * K8 + kc
                n0 = NB * m + 2 * np_
                cb0 = 8 * n0 + kc
                cb1 = 8 * (n0 + 1) + kc
                mbd[pair, 0:WPX, 0:S49] = M[cb0].T
                mbd[pair, WPX:2 * WPX, S49:2 * S49] = M[cb1].T
                idx[pair, 0] = pix[cb0]
                idx[pair, 1] = pix[cb1]
        mbd_sb = np.ascontiguousarray(
            mbd.transpose(1, 0, 2).reshape(128, PAIRS * 2 * S49)
        ).astype(BF16)
        # Host-side window gather (the walrus build here cannot compile the
        # GPSIMD library reload that DMAGatherAnt needs). Layout matches the
        # on-chip lhsT slicing: [c_lo, np, c_hi, j_local].
        flat = idx.reshape(NIDX)
        G = featT[flat]                                       # [8192, 256] bf16
        G = G.reshape(NPAIR, CHUNK, 2, 128).transpose(3, 0, 2, 1)
        gsh = np.ascontiguousarray(G.reshape(128, NPAIR * 2 * CHUNK))
        per_core.append({"mbd": mbd_sb, "gsh": gsh})
    return shared, per_core


# ------------------------------------------------------------ device build
def _build_nc(with_lib=False):
    # with_lib: emit the GPSIMD library-reload for CoreSim (which models
    # library residency). The walrus build here rejects the reload pseudo-op
    # ("ISA wrong length"), and the NEFF path runs the gather fine without
    # it, so hardware builds skip it.
    nc = bass.Bass("TRN2", target_bir_lowering=False, debug=False,
                   num_devices=N_CORES, dynamic_dma_scratch_size=32768)
    dt = mybir.dt
    wsb = nc.dram_tensor("wsb", [128, K8 * 2 * C], dt.bfloat16, kind="ExternalInput").ap()
    mbd = nc.dram_tensor("mbd", [128, PAIRS * 2 * S49], dt.bfloat16, kind="ExternalInput").ap()
    gsh = nc.dram_tensor("gsh", [128, NPAIR * 2 * CHUNK], dt.bfloat16, kind="ExternalInput").ap()
    brow = nc.dram_tensor("brow", [1, C], dt.bfloat16, kind="ExternalInput").ap()
    ones = nc.dram_tensor("ones", [1, 2 * S49], dt.bfloat16, kind="ExternalInput").ap()
    out = nc.dram_tensor("out", [NB, S49, C], dt.float32, kind="ExternalOutput").ap()

    if with_lib:
        nc.gpsimd.load_library(library_config.mlp)

    with tile.TileContext(nc) as tc:
        with (
            tc.tile_pool(name="const", bufs=1) as const,
            tc.tile_pool(name="g", bufs=NPAIR) as gpool,
            tc.tile_pool(name="fsb", bufs=6) as fsb_pool,
            tc.tile_pool(name="fps", bufs=4, space="PSUM") as fps_pool,
            tc.tile_pool(name="ops", bufs=2, space="PSUM") as ops_pool,
            tc.tile_pool(name="osb", bufs=3) as osb_pool,
        ):
            w_sb = const.tile([128, K8 * 2 * C], dt.bfloat16)
            nc.sync.dma_start(w_sb[:], wsb[:])
            mbd_sb = const.tile([128, PAIRS * 2 * S49], dt.bfloat16)
            nc.sync.dma_start(mbd_sb[:], mbd[:])
            brow_sb = const.tile([1, C], dt.bfloat16)
            nc.sync.dma_start(brow_sb[:], brow[:])
            ones_sb = const.tile([1, 2 * S49], dt.bfloat16)
            nc.sync.dma_start(ones_sb[:], ones[:])

            # One window-chunk DMA per box pair (np-major order): each outer
            # iteration below depends on exactly one chunk tile, so compute
            # overlaps the remaining window traffic.
            g_tiles = []
            for np_ in range(NPAIR):
                g_sb = gpool.tile([128, 2, CHUNK], dt.bfloat16)
                # scalar-engine HWDGE ring: window chunks bypass the const
                # DMAs queued on the sync-engine ring.
                nc.scalar.dma_start(
                    g_sb[:, :, :].rearrange("p a b -> p (a b)"),
                    gsh[:, np_ * 2 * CHUNK:(np_ + 1) * 2 * CHUNK],
                )
                g_tiles.append(g_sb)

            for np_ in range(NPAIR):
                g_sb = g_tiles[np_]
                out_ps = ops_pool.tile([128, C], dt.float32)
                for kc in range(K8):
                    pair = np_ * K8 + kc
                    f_ps = fps_pool.tile([128, C], dt.float32)
                    for c_hi in range(2):
                        nc.tensor.matmul(
                            f_ps[:, :],
                            lhsT=g_sb[:, c_hi, kc * 128:(kc + 1) * 128],
                            rhs=w_sb[:, (kc * 2 + c_hi) * C:(kc * 2 + c_hi + 1) * C],
                            start=(c_hi == 0),
                            stop=(c_hi == 1),
                        )
                    f_sb = fsb_pool.tile([128, C], dt.bfloat16)
                    nc.vector.tensor_copy(f_sb[:, :], f_ps[:, :])
                    nc.tensor.matmul(
                        out_ps[0:2 * S49, :],
                        lhsT=mbd_sb[:, pair * 2 * S49:(pair + 1) * 2 * S49],
                        rhs=f_sb[:, :],
                        start=(kc == 0),
                        stop=False,
                    )
                nc.tensor.matmul(
                    out_ps[0:2 * S49, :],
                    lhsT=ones_sb[0:1, :],
                    rhs=brow_sb[0:1, :],
                    start=False,
                    stop=True,
                )
                o_sb = osb_pool.tile([128, C], dt.float32)
                nc.scalar.activation(
                    o_sb[0:2 * S49, :], out_ps[0:2 * S49, :],
                    mybir.ActivationFunctionType.Relu,
                )
                dst = out[2 * np_:2 * np_ + 2, :, :].rearrange("a b c -> (a b) c")
                nc.sync.dma_start(dst, o_sb[0:2 * S49, :])
    _split_multi_waits(nc)
    return nc


_NC_CACHE = None


def _get_nc():
    global _NC_CACHE
    if _NC_CACHE is None:
        _NC_CACHE = _build_nc()
    return _NC_CACHE


def make_in_maps(features, boxes, w_fuse, b_fuse):
    shared, per_core = _prep(features, boxes, w_fuse, b_fuse)
    return [{**shared, **pc} for pc in per_core]


def kernel(features, boxes, w_fuse, b_fuse):
    in_maps = make_in_maps(features, boxes, w_fuse, b_fuse)
    nc = _get_nc()
    res = bass_utils.run_bass_kernel_spmd(
        nc, in_maps, core_ids=list(range(N_CORES)), trace=False
    )
    parts = [res.results[m]["out"] for m in range(N_CORES)]   # each [16, 49, 256]
    full = np.concatenate(parts, axis=0)                      # [128, 49, 256]
    out = full.transpose(0, 2, 1).reshape(N_BOXES, C, OUT, OUT)
    return np.ascontiguousarray(out.astype(np.float32))



# revision 30
# speedup vs baseline: 1.7884x; 1.7884x over previous
"""ContextBranch (context-RoI pooling + 1x1-conv fusion) on 8 Trainium2 cores.

Problem: for each of N=128 boxes, pool the 8 surrounding context cells
(3x3 grid minus center) from a [256, 64, 64] feature map with ROIAlignV2
(7x7 output, sampling_ratio 2), concatenate the 8 pooled chunks into 2048
channels, apply a 1x1 conv (2048->256) + bias + ReLU.

Sharding: box-parallel (no collectives). The reference reshapes the
offset-major pooled [8N,...] to [N, 2048, ...], so output box n consumes
offset-major context rows [8n, 8n+8) == rows [128m, 128m+128) for core m
(boxes n in [16m, 16m+16)). Feature map and fusion weights replicated.

Device algorithm (per core), interp-FIRST to keep every matmul streaming
at full 128-partition output width and to keep the PSUM->SBUF copy
traffic minimal:

  1. ROIAlignV2 is separable; all 14x14 bilinear samples of one context
     cell live in a 7x7 feature-map window (tap span <= 6.5*bin <= 4.26
     px -> 7 px/axis). Host collapses interp+pooling into a dense
     M_cb [49 win-px, 49 out-px] per context box and gathers the window
     pixel vectors TRANSPOSED: G [49 win-px, 128 cb, 256 c] (bf16).
     G and M are streamed together chunk-by-chunk in one DRAM tensor.
  2. Stage A (interp): pooledT[c_half][128 c, 49 s] = G_cb^T @ M_cb per
     (cbox, c-half) - single K=49 matmul per 49-col output slice, no
     accumulation. PSUM -> SBUF (bf16) copies split across DVE (half 0)
     and Act (half 1).
  3. Stage B (fusion): out[o_chunk 128, (j,s) 49-slice] accumulates 16
     matmuls (8 kc x 2 c-half), lhsT = W chunk [128 c, 128 o] - full
     128x128 PE utilization. Emitted in two passes (kc 0-3, kc 4-7) so
     the two W stream-in halves unblock progressively.
  4. Bias+ReLU fused into one op per out tile (DVE tensor_scalar add+max
     for o-chunk 0, Act activation Relu+bias for o-chunk 1), bf16 out,
     DMA'd per (o_chunk, box-group).

A few dozen warm-up matmuls on a zeroed tile run while the first DMAs
land, so the PE p-state ramp (0.65/1.2 GHz for the first ~3us of busy
time) is spent before real work arrives.
"""

import numpy as np
import ml_dtypes

import concourse.bass as bass
import concourse.tile as tile
from concourse import mybir
from concourse import bass_utils
from concourse.vector_clock import ScopedClock

# ---------------------------------------------------------------- constants
OUT = 7          # output size
SR = 2           # sampling ratio
SCALE = 1.0 / 16.0
H = W = 64
C = 256
N_BOXES = 128
N_CORES = 8
NB = N_BOXES // N_CORES   # 16 boxes per core
K8 = 8                    # context offsets
WIN = 7                   # window side (tap span <= 4.26 px -> 7 suffices)
WPX = WIN * WIN           # 49 window pixels
S49 = OUT * OUT           # 49 pooled positions
NCB = NB * K8             # 128 context boxes per core
CB_COLS = C + S49         # 305 gm columns per context box (G block + M block)

# gather/M stream chunking, in CONTEXT BOXES (sum = 128 = 16 boxes x 8).
# Small first chunks get the PE started early; the tail rides behind the
# compute pipeline.
CHUNK_CBS = [8, 8, 8, 16, 16, 16, 16, 16, 24]
NWARM = 44                # PE warm-up matmuls (p-state ramp coverage)
B01_LAG = 2               # boxes between A(j) and B01(j-LAG): covers copy latency
B23_LAG = 4               # boxes between A(j) and B23(j-LAG): covers wsb half 2
COPY_SPLIT = False        # split PSUM->SBUF copies into kc halves
EMIT_MODE = "oc"          # il: interleaved w/ lags; flat: A*, B01*, B23*; box: A,B per box
MANUAL_ORDER = True       # stamp instructions with tile_wait_until pseudo-times
OC1_LAG = 3               # extra lag for B(.,oc1) in "oc" emit mode
FIN_LAG = 0               # boxes between a half-group's last B and its ReLU/DMA
FINISH_HALVES = False     # False: one ReLU pair + one DMA per group
DMA_ORDER = ["g0", "g1", "w0", "g2", "g3", "w1", "b",
             "g4", "g5", "g6", "g7", "g8"]
GROUP_BOXES = 8           # boxes per output PSUM tile (8 or 4)
PPS_BUFS = 4              # pooled PSUM pool depth (banks)
OPS_BUFS = 4              # out PSUM pool depth (banks)

BF16 = ml_dtypes.bfloat16


# ------------------------------------------------------- tile drain patch
def _patched_drain_and_barrier(self, tick_clock, wait_clock):
    # The walrus build in this environment rejects >1 sync wait on a Drain
    # ("Too many sync wait commands"), but Tile's kernel-tail drain carries
    # one wait per live semaphore. Split into chained single-wait drains on
    # the same engine, which is semantically identical.
    nc = self.nc
    drain_bi = nc.sync.drain()
    inst = drain_bi.ins
    wait_clock.add_sem_waits(inst, ScopedClock({None: tick_clock.global_clock}))
    si = inst.sync_info
    waits = list(si.on_wait) if si is not None else []
    if len(waits) > 1:
        inst.sync_info = mybir.SyncInfo(on_wait=[waits[0]], on_update=[])
        for w in waits[1:]:
            d2 = nc.sync.drain()
            d2.ins.sync_info = mybir.SyncInfo(on_wait=[w], on_update=[])

    nc.all_engine_barrier()
    assert self.sems is not None
    popped = nc._tile_sem_poison_stack.pop()
    assert popped is self._sem_poison
    nc.clear_and_free_semaphores(list(self.sems.allocated().values()))
    nc.all_engine_barrier()


tile.TileContext._drain_and_barrier = _patched_drain_and_barrier

_ws_counter = [0]


def _split_multi_waits(nc):
    """Walrus here allows only ONE sync wait per instruction. For every
    instruction carrying N>1 waits, hoist N-1 of them onto injected NoOps on
    the same engine immediately before it (same-engine program order makes
    this semantically identical)."""
    for f in nc.m.functions:
        for blk in f.blocks:
            new_insts = []
            for inst in blk.instructions:
                si = getattr(inst, "sync_info", None)
                waits = list(si.on_wait) if si is not None else []
                if len(waits) > 1:
                    for w in waits[:-1]:
                        _ws_counter[0] += 1
                        nop = mybir.InstNoOp(
                            name=f"I-waitsplit-{_ws_counter[0]}", ins=[], outs=[]
                        )
                        nop.engine = inst.engine
                        nop.sync_info = mybir.SyncInfo(on_wait=[w], on_update=[])
                        nc.register_instruction(nop)
                        new_insts.append(nop)
                    inst.sync_info = mybir.SyncInfo(
                        on_wait=[waits[-1]], on_update=list(si.on_update)
                    )
                new_insts.append(inst)
            blk.instructions = new_insts


# ------------------------------------------------------------- host math
def _context_boxes(boxes):
    """[N,4] -> [8, N, 4] context cells, offset-major (reference order)."""
    boxes = boxes.astype(np.float32)
    x1, y1, x2, y2 = boxes[:, 0], boxes[:, 1], boxes[:, 2], boxes[:, 3]
    w = (x2 - x1) / np.float32(3.0)
    h = (y2 - y1) / np.float32(3.0)
    offs = []
    for i in range(3):
        for j in range(3):
            if i == 1 and j == 1:
                continue
            dx = j * w
            dy = i * h
            offs.append(np.stack([x1 + dx, y1 + dy, x1 + dx + w, y1 + dy + h], axis=1))
    return np.stack(offs, axis=0)


def _axis_weights(lo_c, hi_c, size):
    """Per-axis pooled interp weights for one axis of all B context boxes.

    lo_c, hi_c: [B] box edge coords (image space). Returns (orig [B] int,
    Wax [B, 7, WIN] fp32) with pooling (x0.5) folded in.
    """
    B = lo_c.shape[0]
    start = lo_c * np.float32(SCALE) - np.float32(0.5)
    end = hi_c * np.float32(SCALE) - np.float32(0.5)
    bw = (end - start) / np.float32(OUT)
    j = np.arange(OUT * SR)
    t = (j // SR + ((j % SR) + np.float32(0.5)) / np.float32(SR)).astype(np.float32)
    pos = start[:, None] + t[None, :] * bw[:, None]          # [B, 14]
    valid = (pos >= -1.0) & (pos <= size)
    pc = np.clip(pos, np.float32(0.0), np.float32(size - 1))
    lo = np.clip(np.floor(pc), 0.0, size - 2).astype(np.int64)
    f = (pc - lo.astype(np.float32)).astype(np.float32)
    wl = ((1.0 - f) * valid).astype(np.float32)
    wh = (f * valid).astype(np.float32)
    orig = np.clip(lo.min(axis=1), 0, size - WIN)            # [B]
    rel = lo - orig[:, None]                                 # [B, 14] in [0, WIN-2]
    assert rel.min() >= 0 and rel.max() <= WIN - 2
    Wax = np.zeros((B, OUT, WIN), np.float32)
    bi = np.arange(B)
    for jj in range(OUT * SR):
        g = jj // SR
        Wax[bi, g, rel[:, jj]] += 0.5 * wl[:, jj]
        Wax[bi, g, rel[:, jj] + 1] += 0.5 * wh[:, jj]
    return orig, Wax


def _prep(features, boxes, w_fuse, b_fuse):
    """All host-side layout/index prep. Returns (shared dict, per-core list)."""
    features = np.asarray(features, np.float32)
    boxes = np.asarray(boxes, np.float32)
    w_fuse = np.asarray(w_fuse, np.float32)
    b_fuse = np.asarray(b_fuse, np.float32)

    cb = _context_boxes(boxes).reshape(K8 * N_BOXES, 4)      # [1024, 4]
    B = cb.shape[0]
    ox, Wx = _axis_weights(cb[:, 0], cb[:, 2], W)            # x axis
    oy, Wy = _axis_weights(cb[:, 1], cb[:, 3], H)            # y axis

    # M[b, s=(ph,pw), p=(iy,ix)] = Wy[b,ph,iy] * Wx[b,pw,ix]
    M = (Wy[:, :, None, :, None] * Wx[:, None, :, None, :]).reshape(B, S49, WPX)

    # gather pixel index of window pixel p=(iy,ix) of cbox b
    iy, ix = np.meshgrid(np.arange(WIN), np.arange(WIN), indexing="ij")
    pix = ((oy[:, None, None] + iy) * W + (ox[:, None, None] + ix)).reshape(B, WPX)
    assert pix.min() >= 0 and pix.max() < H * W

    featT = np.ascontiguousarray(features.reshape(C, H * W).T).astype(BF16)

    # fusion weights, oc-major: wsb[p, ((oc*K8+kc)*2+h)*128 + ol] =
    # w_fuse[oc*128+ol, kc*256 + h*128 + p] - so each oc's weights are one
    # contiguous DMA half
    w5 = w_fuse.reshape(2, 128, K8, 2, 128)                  # [oc, ol, kc, h, p]
    wsb = np.ascontiguousarray(
        w5.transpose(4, 0, 2, 3, 1).reshape(128, K8 * 2 * 2 * 128)
    ).astype(BF16)
    bsrc = np.ascontiguousarray(b_fuse.reshape(2, 128).T).astype(np.float32)
    shared = {"wsb": wsb, "bsrc": bsrc}

    per_core = []
    for m in range(N_CORES):
        cbs = np.arange(128 * m, 128 * m + NCB)              # local l = 8j + kc
        G = featT[pix[cbs].reshape(-1)]                      # [128*49, 256] bf16
        G = G.reshape(NCB, WPX, C).transpose(1, 0, 2)        # [49 px, 128 cb, 256]
        Mc = M[cbs].transpose(2, 0, 1).astype(BF16)          # [49 px, 128 cb, 49 s]
        parts = []
        c0 = 0
        for ncb in CHUNK_CBS:
            parts.append(G[:, c0:c0 + ncb, :].reshape(WPX, -1))
            parts.append(Mc[:, c0:c0 + ncb, :].reshape(WPX, -1))
            c0 += ncb
        gm = np.ascontiguousarray(np.concatenate(parts, axis=1))
        per_core.append({"gm": gm})
    return shared, per_core


LABELS = {}   # instruction name -> human label (profiling aid)


def _lab(bi, label):
    try:
        LABELS[bi.ins.name] = label
    except Exception:
        pass
    return bi


# ------------------------------------------------------------ device build
def _chunk_layout():
    """Per-CBOX (chunk tile index, G col offset, M col offset) in gm."""
    cb_loc = []
    chunk_cols = []
    for ncb in CHUNK_CBS:
        gcols = ncb * C
        chunk_cols.append(ncb * CB_COLS)
        for i in range(ncb):
            cb_loc.append((len(chunk_cols) - 1, i * C, gcols + i * S49))
    return cb_loc, chunk_cols


def _build_nc():
    nc = bass.Bass("TRN2", target_bir_lowering=False, debug=False,
                   num_devices=N_CORES)
    dt = mybir.dt
    cb_loc, chunk_cols = _chunk_layout()
    gm_total = sum(chunk_cols)
    gm = nc.dram_tensor("gm", [WPX, gm_total], dt.bfloat16, kind="ExternalInput").ap()
    wsb = nc.dram_tensor("wsb", [128, K8 * 2 * 2 * 128], dt.bfloat16,
                         kind="ExternalInput").ap()
    bsrc = nc.dram_tensor("bsrc", [128, 2], dt.float32, kind="ExternalInput").ap()
    ngrp = NB // GROUP_BOXES
    hb = GROUP_BOXES // 2
    out = nc.dram_tensor("out", [ngrp, 128, 2, GROUP_BOXES, S49], dt.bfloat16,
                         kind="ExternalOutput").ap()

    relu = mybir.ActivationFunctionType.Relu
    add_op = mybir.AluOpType.add
    max_op = mybir.AluOpType.max

    with tile.TileContext(nc) as tc:
        with (
            tc.tile_pool(name="pps", bufs=PPS_BUFS, space="PSUM") as pps_pool,
            tc.tile_pool(name="ops", bufs=OPS_BUFS, space="PSUM") as ops_pool,
            tc.tile_pool(name="osb", bufs=4) as osb_pool,
            tc.tile_pool(name="consts", bufs=1) as consts,
        ):
            # ---- DMA issue order (sync queue == transfer order) ----
            gm_tiles = []
            col0 = 0
            for t in range(len(CHUNK_CBS)):
                g_sb = consts.tile([WPX, chunk_cols[t]], dt.bfloat16,
                                   name=f"gmt{t}")
                gm_tiles.append((g_sb, col0))
                col0 += chunk_cols[t]
            w_sb = consts.tile([128, K8 * 2 * 2 * 128], dt.bfloat16)
            b_sb = consts.tile([128, 2], dt.float32)

            def dma_chunk(t):
                g_sb, c0 = gm_tiles[t]
                nc.sync.dma_start(g_sb[:, :], gm[:, c0:c0 + chunk_cols[t]])

            # stream order: a couple of box chunks to get stage A going,
            # the W halves (oc-major) interleaved so B(.,oc0) unlocks early,
            # then the remaining gather stream.
            def dma_wsb(half):
                nc.sync.dma_start(w_sb[:, half * 2048:(half + 1) * 2048],
                                  wsb[:, half * 2048:(half + 1) * 2048])

            for item in DMA_ORDER:
                if item == "w0":
                    dma_wsb(0)
                elif item == "w1":
                    dma_wsb(1)
                elif item == "b":
                    nc.sync.dma_start(b_sb[:, :], bsrc[:, :])
                else:
                    dma_chunk(int(item[1:]))

            # ---- PE warm-up (p-state ramp) ----
            warm_sb = consts.tile([1, 64], dt.bfloat16)
            nc.gpsimd.memset(warm_sb[:, :], 0)
            warm_ps = ops_pool.tile([64, 64], dt.float32, name="o_ps")
            for _ in range(NWARM):
                nc.tensor.matmul(warm_ps[:, :], lhsT=warm_sb[0:1, :],
                                 rhs=warm_sb[0:1, :], start=True, stop=True)

            # pooled bf16 staging in SBUF, one big tile per c-half
            p_sb = [consts.tile([128, NB * K8 * S49], dt.bfloat16, name=f"psb{h}")
                    for h in range(2)]
            o_tiles = {}   # (g, oc) -> PSUM out tile

            def stage_a(j):
                nparts = 2 if COPY_SPLIT else 1
                psz = (K8 * S49) // nparts
                for h in range(2):
                    pooled = pps_pool.tile([128, K8 * S49], dt.float32,
                                           name="pooled")
                    for part in range(nparts):
                        for kc in range((K8 // nparts) * part,
                                        (K8 // nparts) * (part + 1)):
                            t, gc, mc = cb_loc[j * K8 + kc]
                            g_sb, _ = gm_tiles[t]
                            _lab(nc.tensor.matmul(
                                pooled[:, kc * S49:(kc + 1) * S49],
                                lhsT=g_sb[:, gc + h * 128:gc + h * 128 + 128],
                                rhs=g_sb[:, mc:mc + S49],
                                start=True, stop=True,
                            ), f"A(j={j},h={h},kc={kc})")
                        # copy as soon as the matmuls land (DVE: h0, Act: h1)
                        src = pooled[:, part * psz:(part + 1) * psz]
                        dst = p_sb[h][:, j * K8 * S49 + part * psz:
                                      j * K8 * S49 + (part + 1) * psz]
                        if h == 0:
                            _lab(nc.vector.tensor_copy(dst, src),
                                 f"cp(j={j},h0,{part})")
                        else:
                            _lab(nc.scalar.copy(dst, src),
                                 f"cp(j={j},h1,{part})")

            def stage_b(j, kc_lo, kc_hi, ocs=(0, 1)):
                g = j // GROUP_BOXES
                for oc in ocs:
                    key = (g, oc)
                    if key not in o_tiles:
                        o_tiles[key] = ops_pool.tile(
                            [128, GROUP_BOXES * S49], dt.float32, name="o_ps")
                    o_ps = o_tiles[key]
                    jj = j % GROUP_BOXES
                    sl = o_ps[:, jj * S49:(jj + 1) * S49]
                    for kc in range(kc_lo, kc_hi):
                        for h in range(2):
                            widx = (oc * K8 + kc) * 2 + h
                            _lab(nc.tensor.matmul(
                                sl,
                                lhsT=w_sb[:, widx * 128:(widx + 1) * 128],
                                rhs=p_sb[h][:, j * K8 * S49 + kc * S49:
                                            j * K8 * S49 + (kc + 1) * S49],
                                start=(kc == 0 and h == 0),
                                stop=(kc == K8 - 1 and h == 1),
                            ), f"B(j={j},oc={oc},kc={kc},h={h})")

            o_sbs = {}

            def finish_group_whole(g):
                # bias+ReLU fused; oc 0 on DVE, oc 1 on Act, one DMA ships
                # the whole group's output.
                gw = GROUP_BOXES * S49
                o_sb = osb_pool.tile([128, 2 * gw], dt.bfloat16, name="o_sb")
                _lab(nc.vector.tensor_scalar(
                    o_sb[:, 0:gw], o_tiles[(g, 0)][:, :], b_sb[:, 0:1], 0.0,
                    add_op, max_op,
                ), f"relu(g={g},oc=0)")
                _lab(nc.scalar.activation(
                    o_sb[:, gw:2 * gw], o_tiles[(g, 1)][:, :], relu,
                    bias=b_sb[:, 1:2]), f"relu(g={g},oc=1)")
                dst = out[g].rearrange("o a j s -> o (a j s)")
                _lab(nc.sync.dma_start(dst, o_sb[:, :]), f"outdma(g={g})")

            def finish_half(g, p):
                # bias+ReLU fused per half-group piece; oc 0 on DVE, oc 1 on
                # Act; one contiguous DMA per piece keeps the final chain
                # short and lets piece 0 ship while piece 1 still computes.
                hw_ = (GROUP_BOXES // 2) * S49
                if g not in o_sbs:
                    o_sbs[g] = osb_pool.tile([128, 4 * hw_], dt.bfloat16,
                                             name="o_sb")
                o_sb = o_sbs[g]
                base = p * 2 * hw_
                _lab(nc.vector.tensor_scalar(
                    o_sb[:, base:base + hw_],
                    o_tiles[(g, 0)][:, p * hw_:(p + 1) * hw_], b_sb[:, 0:1],
                    0.0, add_op, max_op,
                ), f"relu(g={g},p={p},oc=0)")
                _lab(nc.scalar.activation(
                    o_sb[:, base + hw_:base + 2 * hw_],
                    o_tiles[(g, 1)][:, p * hw_:(p + 1) * hw_], relu,
                    bias=b_sb[:, 1:2]), f"relu(g={g},p={p},oc=1)")
                dst = out[g][:, :, p * hb:(p + 1) * hb, :]
                srcv = o_sb[:, base:base + 2 * hw_].rearrange(
                    "o (a j s) -> o a j s", a=2, j=hb, s=S49)
                _lab(nc.sync.dma_start(dst, srcv), f"outdma(g={g},p={p})")

            # ---- pipeline emission ----
            _step = [0]

            def _stamp():
                # monotone pseudo-timestamps (1 "ms" apart) dominate the
                # scheduler's modeled ready times, making emission order the
                # de-facto engine order (see tile_set_cur_wait docstring).
                _step[0] += 1
                return tc.tile_wait_until(_step[0], enable=MANUAL_ORDER)

            def maybe_finish(jdone):
                if jdone < 0:
                    return
                if FINISH_HALVES:
                    if (jdone + 1) % (GROUP_BOXES // 2) == 0:
                        g, p = divmod(jdone // (GROUP_BOXES // 2), 2)
                        finish_half(g, p)
                elif (jdone + 1) % GROUP_BOXES == 0:
                    finish_group_whole(jdone // GROUP_BOXES)

            if EMIT_MODE == "il":
                for j in range(NB):
                    stage_a(j)
                    if j >= B01_LAG:
                        stage_b(j - B01_LAG, 0, 4)
                    if j >= B23_LAG:
                        stage_b(j - B23_LAG, 4, K8)
                        maybe_finish(j - B23_LAG)
                for j in range(NB - B01_LAG, NB):
                    stage_b(j, 0, 4)
                for j in range(NB - B23_LAG, NB):
                    stage_b(j, 4, K8)
                    maybe_finish(j)
            elif EMIT_MODE == "flat":
                for j in range(NB):
                    stage_a(j)
                for j in range(NB):
                    stage_b(j, 0, 4)
                for j in range(NB):
                    stage_b(j, 4, K8)
                    maybe_finish(j)
            elif EMIT_MODE == "box":
                for j in range(NB):
                    stage_a(j)
                    stage_b(j, 0, K8)
                    maybe_finish(j)
            elif EMIT_MODE == "mono":
                for j in range(NB):
                    with _stamp():
                        stage_a(j)
                    if j >= B01_LAG:
                        with _stamp():
                            stage_b(j - B01_LAG, 0, K8)
                            maybe_finish(j - B01_LAG)
                for j in range(NB - B01_LAG, NB):
                    with _stamp():
                        stage_b(j, 0, K8)
                        maybe_finish(j)
            elif EMIT_MODE == "oc":
                done1 = set()

                def fin_oc(k):
                    done1.add(k)
                    if FINISH_HALVES:
                        hb_ = GROUP_BOXES // 2
                        lo = (k // hb_) * hb_
                        if all(x in done1 for x in range(lo, lo + hb_)):
                            g, p = divmod(lo // hb_, 2)
                            finish_half(g, p)
                    else:
                        lo = (k // GROUP_BOXES) * GROUP_BOXES
                        if all(x in done1
                               for x in range(lo, lo + GROUP_BOXES)):
                            finish_group_whole(lo // GROUP_BOXES)

                for j in range(NB):
                    with _stamp():
                        stage_a(j)
                    if j >= B01_LAG:
                        with _stamp():
                            stage_b(j - B01_LAG, 0, K8, ocs=(0,))
                    if j >= OC1_LAG:
                        with _stamp():
                            stage_b(j - OC1_LAG, 0, K8, ocs=(1,))
                            if FIN_LAG == 0:
                                fin_oc(j - OC1_LAG)
                    if FIN_LAG > 0 and j >= OC1_LAG + FIN_LAG:
                        with _stamp():
                            fin_oc(j - OC1_LAG - FIN_LAG)
                for j in range(NB - B01_LAG, NB):
                    with _stamp():
                        stage_b(j, 0, K8, ocs=(0,))
                for j in range(NB - OC1_LAG, NB):
                    with _stamp():
                        stage_b(j, 0, K8, ocs=(1,))
                        if FIN_LAG == 0:
                            fin_oc(j)
                if FIN_LAG > 0:
                    for j in range(NB - OC1_LAG - FIN_LAG, NB):
                        with _stamp():
                            fin_oc(j)
            else:
                raise ValueError(EMIT_MODE)

    _split_multi_waits(nc)
    return nc


_NC_CACHE = None


def _get_nc():
    global _NC_CACHE
    if _NC_CACHE is None:
        _NC_CACHE = _build_nc()
    return _NC_CACHE


def make_in_maps(features, boxes, w_fuse, b_fuse):
    shared, per_core = _prep(features, boxes, w_fuse, b_fuse)
    return [{**shared, **pc} for pc in per_core]


def kernel(features, boxes, w_fuse, b_fuse):
    in_maps = make_in_maps(features, boxes, w_fuse, b_fuse)
    nc = _get_nc()
    res = bass_utils.run_bass_kernel_spmd(
        nc, in_maps, core_ids=list(range(N_CORES)), trace=False
    )
    parts = []
    for m in range(N_CORES):
        r = np.asarray(res.results[m]["out"]).astype(np.float32)
        # [g, ol, oc, jb, s] -> [g, jb, oc, ol, s]
        parts.append(r.transpose(0, 3, 2, 1, 4).reshape(NB, C, S49))
    full = np.concatenate(parts, axis=0)                      # [128, 256, 49]
    return np.ascontiguousarray(full.reshape(N_BOXES, C, OUT, OUT))
